# revision 81
# baseline (speedup 1.0000x reference)
"""Trainium2 Bass kernel for MultiHeadCrossAttention.

Problem (hardcoded): B=2, T_q=1024, T_kv=4096, C=1024, H=16, D=64.
  q = x_q @ W_q ; k,v = split(x_kv @ W_kv) ; att = softmax(mask(q k^T / sqrt(D)))
  out = (att v) @ W_c

Sharding over 8 NeuronCores: data-parallel over B (2) x tensor-parallel over
head groups (4 groups of 4 heads). Each core computes a partial c_proj output
(row-parallel W_c); the host sums the 4 head-group partials per batch.

Device-side strategy per core (cost-model-shaped: matmul cost = moving-operand
columns only, stationary loads are free, so keep every moving operand bf16 and
minimize streamed columns):
 - kv tokens are compacted with transposing dma_gathers driven by the kv mask
   (~half the tokens are masked out), padded to a multiple of 128; pad slots
   are neutralized with an additive -100 bias inside the softmax exp. x_q^T is
   built the same way (identity-index transposing gather) so the PE never
   transposes inputs.
 - Scores are computed transposed (s^T[k_tok, q_tok]) so the pad-mask bias is
   per-partition and one exp activation covers a head pair ([128, 2x512]).
 - att@v swaps operands: exp(p) tiles become the *stationary* operand and the
   65-column augmented v (64 v cols + ones column -> softmax denominator) is
   the moving operand, cutting att@v PE time ~8x vs streaming p.
 - y is normalized on DVE (per-partition 1/den), PE-transposed back to [d, q]
   and fed to c_proj. All matmul operands are bf16; PSUM accumulates fp32.
 - Schedule: stream A=(pr0,qc0) runs live-lagged while B/C/D exps are pre-run
   into held SBUF p tiles; their att@v replays + c_proj fill PE while the
   activation engine drains the remaining exps.
"""

import os
import numpy as np
import ml_dtypes

import concourse.bass as bass
import concourse.bacc as bacc
import concourse.mybir as mybir
import concourse.tile as tile
from concourse.bass_utils import run_bass_kernel_spmd

B, TQ, TKV, C, H, D = 2, 1024, 4096, 1024, 16, 64
NCORES = 8
GROUPS = 4          # head groups (tensor-parallel)
GH = H // GROUPS    # 4 heads per group
GC = GH * D         # 256 cols per group

F32 = mybir.dt.float32
BF16 = mybir.dt.bfloat16
I16 = mybir.dt.int16
EXP = mybir.ActivationFunctionType.Exp
BF16NP = ml_dtypes.bfloat16

LAST_RESULTS = None
_PROG_CACHE = {}


def _chunk_widths(nkt):
    widths = []
    r = nkt
    for w0 in (2, 3):
        if r >= w0 + 2:
            widths.append(w0)
            r -= w0
    while r > 0:
        w = 4 if r >= 4 else r
        widths.append(w)
        r -= w
    return widths


def build_program(nkt: int) -> bass.Bass:
    """One SPMD program, shared by all 8 cores. nkt = padded kv tiles of 128."""
    NKP = nkt * 128
    nc = bacc.Bacc("TRN2", target_bir_lowering=False)

    xq_d = nc.declare_dram_parameter("xq", [TQ, C], BF16, isOutput=False)
    xkv_d = nc.declare_dram_parameter("xkv", [TKV, C], BF16, isOutput=False)
    wq_d = nc.declare_dram_parameter("wq", [128, 8 * GC], BF16, isOutput=False)
    wk_d = nc.declare_dram_parameter("wk", [128, 8 * GC], BF16, isOutput=False)
    wv_d = nc.declare_dram_parameter("wv", [128, 8 * GC], BF16, isOutput=False)
    wc_d = nc.declare_dram_parameter("wc", [128, 2 * C], BF16, isOutput=False)
    ident_d = nc.declare_dram_parameter("ident", [128, 128], BF16, isOutput=False)
    idx_d = nc.declare_dram_parameter(
        "idx", [128, TQ // 16 + NKP // 16], I16, isOutput=False)
    bias_d = nc.declare_dram_parameter("bias", [128, nkt], F32, isOutput=False)
    out_d = nc.declare_dram_parameter("out", [TQ, C], BF16, isOutput=True)

    widths = _chunk_widths(nkt)
    kt0s = np.cumsum([0] + widths[:-1]).tolist()
    STREAMS = [(0, 0), (1, 0), (0, 1), (1, 1)]  # A, B, C, D

    with tile.TileContext(nc) as tc, nc.allow_low_precision(reason="bf16 kernel"):
        with (
            tc.tile_pool(name="consts", bufs=1) as consts,
            tc.tile_pool(name="wpool", bufs=1) as wpool,
            tc.tile_pool(name="kvstore", bufs=1) as kvstore,
            tc.tile_pool(name="xkTp", bufs=5) as xkTp,
            tc.tile_pool(name="ph", bufs=35) as ph,
            tc.tile_pool(name="ynp", bufs=2) as ynp,
            tc.tile_pool(name="rdp", bufs=2) as rdp,
            tc.tile_pool(name="outp", bufs=4) as outp,
            tc.tile_pool(name="ps_misc", bufs=2, space="PSUM") as ps_misc,
            tc.tile_pool(name="ps_s", bufs=2, space="PSUM") as ps_s,
            tc.tile_pool(name="ps_y", bufs=2, space="PSUM") as ps_y,
        ):
            # warmup source tile (also feeds the PE warmup matmuls)
            dmy = consts.tile([128, 512], BF16)
            nc.gpsimd.memset(dmy, 0.0)
            # ---- index tile + first weights on the sync queue, in the order
            # the serial DMA device should serve them; wv/wc/ident follow
            # later from the scalar queue.
            idx_t = consts.tile([128, TQ // 16 + NKP // 16], I16)
            nc.sync.dma_start(out=idx_t, in_=idx_d[:, :])
            idxq_t = idx_t[:, : TQ // 16]
            idxk_t = idx_t[:, TQ // 16 :]
            wq_t = wpool.tile([128, 2, 8, 128], BF16)
            nc.sync.dma_start(out=wq_t[:, 0], in_=wq_d[:, 0:1024])
            wk_t = wpool.tile([128, 2, 8, 128], BF16)
            nc.sync.dma_start(out=wk_t[:, 0], in_=wk_d[:, 0:1024])
            nc.sync.dma_start(out=wq_t[:, 1], in_=wq_d[:, 1024:2048])
            nc.sync.dma_start(out=wk_t[:, 1], in_=wk_d[:, 1024:2048])
            wv_t = wpool.tile([128, 8, GC], BF16)
            nc.sync.dma_start(out=wv_t, in_=wv_d[:, :])
            wc_t = wpool.tile([128, 2, C], BF16)
            nc.sync.dma_start(out=wc_t, in_=wc_d[:, :])
            identity = consts.tile([128, 128], BF16)
            nc.sync.dma_start(out=identity, in_=ident_d[:, :])
            bias_t = consts.tile([128, nkt], F32)
            nc.scalar.dma_start(out=bias_t, in_=bias_d[:, :])
            # pre-warm the Exp activation table while ACT is idle; the extra
            # warm passes also delay the non-critical weight DMA triggers so
            # they enqueue behind the first gathers on the serial DMA device.
            warm = consts.tile([128, 4], F32)
            nc.scalar.activation(out=warm, in_=bias_t[:, 0:4], func=EXP)

            # ---- persistent activations ----
            qT = kvstore.tile([128, 2, TQ], BF16)       # [d of pair, pr, q]
            kT = kvstore.tile([128, 2, NKP], BF16)      # [d of pair, pr, ktok]
            vaug = kvstore.tile([128, nkt, GH * (D + 1)], BF16)
            nc.vector.memset(
                vaug.rearrange("p k (h c) -> p k h c", c=D + 1)[:, :, :, D], 1.0
            )
            yT = kvstore.tile([128, 2, TQ], BF16)       # [d of pair, pr, q]

            # ---- transposing gathers (Pool queue): xq first, then kv chunks
            xq_tiles = [
                kvstore.tile([128, 8, 512], BF16, name=f"xqT{h}") for h in range(2)
            ]
            xk_tiles = [
                xkTp.tile([128, 8, w * 128], BF16, tag="xkT", name="xkT_t")
                for w in widths
            ]

            def gather_xq(half):
                nc.gpsimd.dma_gather(
                    out_ap=xq_tiles[half][:, :, :],
                    in_ap=xq_d[:, :],
                    idxs_ap=idxq_t[:, half * 32 : half * 32 + 32],
                    num_idxs=512,
                    num_idxs_reg=512,
                    elem_size=C,
                    transpose=True,
                )

            def gather_kv(sc):
                kt0, w = kt0s[sc], widths[sc]
                ni = w * 128
                nc.gpsimd.dma_gather(
                    out_ap=xk_tiles[sc][:, :, :ni],
                    in_ap=xkv_d[:, :],
                    idxs_ap=idxk_t[:, kt0 * 8 : kt0 * 8 + ni // 16],
                    num_idxs=ni,
                    num_idxs_reg=ni,
                    elem_size=C,
                    transpose=True,
                )

            gather_xq(0)
            gather_kv(0)
            gather_kv(1)
            gather_kv(2)
            gather_xq(1)
            for sc in range(3, len(widths)):
                gather_kv(sc)

            def qproj(pr, qc):
                qp = ps_misc.tile([128, 512], F32, tag="mm", name="qp")
                for c in range(8):
                    nc.tensor.matmul(
                        qp,
                        wq_t[:, pr, c, :],
                        xq_tiles[qc][:, c, :],
                        start=(c == 0),
                        stop=(c == 7),
                    )
                nc.vector.tensor_copy(
                    out=qT[:, pr, qc * 512 : (qc + 1) * 512], in_=qp
                )

            def kproj(sc):
                kt0, w = kt0s[sc], widths[sc]
                ni = w * 128
                for pr in range(2):
                    kp = ps_misc.tile([128, 512], F32, tag="mm", name="kp")
                    for c in range(8):
                        nc.tensor.matmul(
                            kp[:, :ni],
                            wk_t[:, pr, c, :],
                            xk_tiles[sc][:, c, :ni],
                            start=(c == 0),
                            stop=(c == 7),
                        )
                    nc.vector.tensor_copy(
                        out=kT[:, pr, kt0 * 128 : kt0 * 128 + ni],
                        in_=kp[:, :ni],
                    )

            def vproj(kt):
                # find chunk holding kt
                sc = max(i for i, k0 in enumerate(kt0s) if k0 <= kt)
                j = kt - kt0s[sc]
                vp = ps_misc.tile([128, GC], F32, tag="mm", name="vp")
                for c in range(8):
                    nc.tensor.matmul(
                        vp,
                        xk_tiles[sc][:, c, j * 128 : (j + 1) * 128],
                        wv_t[:, c, :],
                        start=(c == 0),
                        stop=(c == 7),
                    )
                nc.vector.tensor_copy(
                    out=vaug.rearrange("p k (h c) -> p k h c", c=D + 1)[
                        :, kt, :, 0:D
                    ],
                    in_=vp.rearrange("p (h c) -> p h c", c=D),
                )

            held_p = {}   # (stream_idx, kt) -> p tile

            def score_exp(si, kt):
                pr, qc = STREAMS[si]
                s2 = ps_s.tile([128, 2, 512], F32, tag="s", name="s2")
                nc.tensor.matmul(
                    s2[:, 0, :],
                    kT[0:64, pr, kt * 128 : (kt + 1) * 128],
                    qT[0:64, pr, qc * 512 : (qc + 1) * 512],
                    start=True, stop=True,
                )
                nc.tensor.matmul(
                    s2[:, 1, :],
                    kT[64:128, pr, kt * 128 : (kt + 1) * 128],
                    qT[64:128, pr, qc * 512 : (qc + 1) * 512],
                    start=True, stop=True,
                )
                p = ph.tile([128, 2, 512], BF16, tag="p", name="p")
                nc.scalar.activation(
                    out=p, in_=s2, func=EXP,
                    bias=bias_t[:, kt : kt + 1], scale=0.125,
                )
                held_p[(si, kt)] = p

            y_acc = {}

            def attv(si, kt):
                pr, qc = STREAMS[si]
                if kt == 0:
                    y_acc[(si, 0)] = ps_y.tile([128, 4, D + 1], F32, tag="y",
                                               name="y0")
                    y_acc[(si, 1)] = ps_y.tile([128, 4, D + 1], F32, tag="y",
                                               name="y1")
                p = held_p.pop((si, kt))
                for h in range(2):
                    va = vaug[:, kt, (2 * pr + h) * (D + 1) :
                              (2 * pr + h + 1) * (D + 1)]
                    for ch in range(4):
                        # one accumulation group per PSUM bank: start only on
                        # the bank's first write (start zeroes the whole 2KB
                        # zero-region); kt=0 writes of other chunks first-touch
                        # their bytes, later kts accumulate.
                        nc.tensor.matmul(
                            y_acc[(si, h)][:, ch, :],
                            p[:, h, ch * 128 : (ch + 1) * 128],
                            va,
                            start=(kt == 0 and ch == 0),
                            stop=(kt == nkt - 1 and ch == 3),
                        )

            def norm_transpose(si, fused_cproj=False):
                pr, qc = STREAMS[si]
                yn = ynp.tile([128, 4, 128], BF16, tag="yn", name="yn")
                for h in range(2):
                    yh = y_acc.pop((si, h))
                    rden = rdp.tile([128, 4, 1], F32, tag="rden", name="rden")
                    nc.vector.reciprocal(out=rden, in_=yh[:, :, D : D + 1])
                    in0b, rdb = bass.broadcast_tensor_aps(yh[:, :, 0:D], rden)
                    nc.vector.tensor_mul(yn[:, :, h * D : (h + 1) * D],
                                         in0b, rdb)
                for ch in range(4):
                    tpool = ps_s if fused_cproj else ps_misc
                    tp = tpool.tile([128, 128], BF16,
                                    tag="s" if fused_cproj else "mm", name="tp")
                    nc.tensor.transpose(tp, yn[:, ch, :], identity)
                    nc.vector.tensor_copy(
                        out=yT[:, pr, (qc * 4 + ch) * 128 :
                               (qc * 4 + ch + 1) * 128],
                        in_=tp,
                    )
                    if fused_cproj:
                        cproj(qc * 4 + ch, evict_on_act=True)

            def cproj(t, evict_on_act=False):
                ot = outp.tile([128, 2, 512], BF16, tag="ot", name="ot")
                for n in range(2):
                    cp = ps_misc.tile([128, 512], F32, tag="mm", name="cp")
                    for pr in range(2):
                        nc.tensor.matmul(
                            cp,
                            yT[:, pr, t * 128 : (t + 1) * 128],
                            wc_t[:, pr, n * 512 : (n + 1) * 512],
                            start=(pr == 0),
                            stop=(pr == 1),
                        )
                    if evict_on_act and n == 1:
                        nc.scalar.copy(out=ot[:, n, :], in_=cp)
                    else:
                        nc.vector.tensor_copy(out=ot[:, n, :], in_=cp)
                    if evict_on_act:
                        # tail: split DMAs pipeline the last transfers
                        nc.sync.dma_start(
                            out=out_d[t * 128 : (t + 1) * 128,
                                      n * 512 : (n + 1) * 512],
                            in_=ot[:, n, :],
                        )
                if not evict_on_act:
                    nc.sync.dma_start(
                        out=out_d[t * 128 : (t + 1) * 128, :],
                        in_=ot,
                    )

            # ================= schedule =================
            NK = nkt
            LAG_A = min(14, NK - 2)    # stream A att@v lag inside P1
            VS = NK - LAG_A            # vprojs emitted in P1 (kt 0..VS-1)

            # P0: dummy matmuls keep the PE busy through the DMA lead-in so
            # the cost model's pstate ramp is warm when real work arrives.
            for _ in range(20):
                dp = ps_misc.tile([128, 512], F32, tag="mm", name="dp")
                nc.tensor.matmul(dp, dmy[:, 0:128], dmy[:, :],
                                 start=True, stop=True)
            # q projections for qc=0 only (xq0 is the first gather);
            # the qc=1 units are deferred into P1 once xq1 has landed.
            qproj(0, 0)
            qproj(1, 0)

            # deferred qc=1 q-projection, spread in 2-chunk pieces so the
            # in-order PE queue never starves the score->exp stream
            qp_hold = {}

            def qproj_piece(pr, qc, i):
                if i == 0:
                    qp_hold[(pr, qc)] = ps_misc.tile([128, 512], F32,
                                                     tag="mm", name="qp")
                qp = qp_hold[(pr, qc)]
                for c in (2 * i, 2 * i + 1):
                    nc.tensor.matmul(
                        qp,
                        wq_t[:, pr, c, :],
                        xq_tiles[qc][:, c, :],
                        start=(c == 0),
                        stop=(c == 7),
                    )
                if i == 3:
                    nc.vector.tensor_copy(
                        out=qT[:, pr, qc * 512 : (qc + 1) * 512],
                        in_=qp_hold.pop((pr, qc)),
                    )

            # P1: kproj chunks + A/B scores+exps; A att@v live-lagged;
            #     vproj for the first VS tiles.
            done_chunks = set()
            for kt in range(NK):
                sc = max(i for i, k0 in enumerate(kt0s) if k0 <= kt)
                if sc not in done_chunks:
                    done_chunks.add(sc)
                    kproj(sc)
                if kt == 5:
                    qproj(0, 1)
                if kt == 6:
                    qproj(1, 1)
                score_exp(0, kt)
                score_exp(1, kt)
                if kt < VS:
                    vproj(kt)
                if kt >= LAG_A:
                    attv(0, kt - LAG_A)

            # P2: C scores+exps; drain A tail, replay B; rest of vproj.
            b_sched = {}   # iter -> list of B kts
            rem = list(range(NK))
            start_it = LAG_A + 1
            per = max(1, -(-len(rem) // max(1, NK - start_it)))
            it = start_it
            while rem:
                b_sched.setdefault(it, []).extend(rem[:per])
                rem = rem[per:]
                it += 1
            for kt in range(NK):
                score_exp(2, kt)
                if VS + kt < NK:
                    vproj(VS + kt)
                if kt < LAG_A:
                    attv(0, NK - LAG_A + kt)
                if kt == LAG_A:
                    norm_transpose(0)
                for bkt in b_sched.get(kt, []):
                    attv(1, bkt)
            norm_transpose(1)

            # P3: D scores+exps live; replay C compressed (2 kts/iter) so its
            # banks free mid-loop and D's replay can ride live-lagged; c_proj
            # for qc=0 spread as PE filler; final c_proj tiles fused into D's
            # per-chunk epilogue.
            c_done = (NK + 1) // 2          # iter when C replay completes

            d_start = c_done + 1            # first iter allowed to run D att@v
            d_done = set()
            for kt in range(NK):
                score_exp(3, kt)
                for ckt in range(2 * kt, min(2 * kt + 2, NK)):
                    attv(2, ckt)
                if kt == c_done:
                    norm_transpose(2)
                if kt >= d_start:
                    for j in range(2 * (kt - d_start),
                                   min(2 * (kt - d_start) + 2, kt)):
                        if j not in d_done:
                            d_done.add(j)
                            attv(3, j)
                if kt == 2:
                    cproj(0)
                if kt == 6:
                    cproj(1)
                if kt == 11:
                    cproj(2)
                if kt == 15:
                    cproj(3)
            for kt in range(NK):
                if kt not in d_done:
                    attv(3, kt)
            norm_transpose(3, fused_cproj=True)

    nc.compile()
    return nc


def make_in_maps(x_q, x_kv, kv_tok_mask, W_q, W_kv, W_c):
    mask = np.asarray(kv_tok_mask).astype(bool)
    valid = [np.nonzero(mask[b])[0] for b in range(B)]
    nv = [len(v) for v in valid]
    NKP = max(128, -(-max(max(nv), 1) // 128) * 128)
    nkt = NKP // 128

    per_batch = []
    for b in range(B):
        idx = np.zeros(NKP, np.int16)
        idx[: nv[b]] = valid[b].astype(np.int16)
        bias = np.full(NKP, -100.0, np.float32)
        bias[: nv[b]] = 0.0
        wrapped = idx.reshape(NKP // 16, 16).T  # [16, NKP/16]
        idxs_dev = np.ascontiguousarray(np.tile(wrapped, (8, 1)))  # [128, NKP/16]
        bias_dev = np.ascontiguousarray(bias.reshape(nkt, 128).T)  # [128, nkt]
        per_batch.append((idxs_dev, bias_dev))

    idq = np.arange(TQ, dtype=np.int16).reshape(TQ // 16, 16).T
    idxq_dev = np.ascontiguousarray(np.tile(idq, (8, 1)))  # [128, TQ/16]

    f32 = lambda a: np.ascontiguousarray(np.asarray(a, dtype=np.float32))
    bf16 = lambda a: np.ascontiguousarray(np.asarray(a, dtype=np.float32).astype(BF16NP))
    x_qb, x_kvb = bf16(x_q), bf16(x_kv)
    W_qb, W_kvb, W_cb = bf16(W_q), bf16(W_kv), bf16(W_c)
    ident = np.eye(128, dtype=BF16NP)

    def chunkP(w):  # [C, N] -> [128, (C//128)*N] with (p, c, n) layout
        Cd, N = w.shape
        return np.ascontiguousarray(
            w.reshape(Cd // 128, 128, N).transpose(1, 0, 2).reshape(128, -1))

    def chunkPpr(w):  # [C, 256] -> [128, 2*(C//128)*128] with (p, pr, c, n)
        Cd, N = w.shape
        return np.ascontiguousarray(
            w.reshape(Cd // 128, 128, 2, 128).transpose(1, 2, 0, 3)
            .reshape(128, -1))

    in_maps = []
    for core in range(NCORES):
        b, hg = core // GROUPS, core % GROUPS
        cols = slice(hg * GC, (hg + 1) * GC)
        in_maps.append(
            {
                "xq": x_qb[b],
                "xkv": x_kvb[b],
                "wq": chunkPpr(W_qb[:, cols]),
                "wk": chunkPpr(W_kvb[:, cols]),
                "wv": chunkP(W_kvb[:, C + hg * GC : C + (hg + 1) * GC]),
                "wc": chunkP(W_cb[cols, :]),
                "ident": ident,
                "idx": np.ascontiguousarray(
                    np.concatenate([idxq_dev, per_batch[b][0]], axis=1)),
                "bias": per_batch[b][1],
            }
        )
    return in_maps, nkt


def kernel(x_q, x_kv, q_tok_mask, kv_tok_mask, W_q, W_kv, W_c):
    global LAST_RESULTS
    in_maps, nkt = make_in_maps(x_q, x_kv, kv_tok_mask, W_q, W_kv, W_c)
    if nkt not in _PROG_CACHE:
        _PROG_CACHE[nkt] = build_program(nkt)
    nc = _PROG_CACHE[nkt]
    want_trace = bool(os.environ.get("KBENCH_TRACE"))
    if want_trace:
        try:
            from antenv.axon_hooks import get_axon_ntff_profile_hook  # noqa: F401
        except ImportError:
            want_trace = False  # axon client lacks the NTFF hook
    res = run_bass_kernel_spmd(
        nc,
        in_maps,
        core_ids=list(range(NCORES)),
        trace=want_trace,
    )
    LAST_RESULTS = res
    outs = [np.asarray(res.results[c]["out"], dtype=np.float32)
            for c in range(NCORES)]
    full = np.zeros((B, TQ, C), np.float32)
    for b in range(B):
        full[b] = outs[GROUPS * b]
        for g in range(1, GROUPS):
            full[b] += outs[GROUPS * b + g]
    return full
